# revision 19
# baseline (speedup 1.0000x reference)
"""EnergyAttention Trainium2 kernel (8-core SPMD, head/q hybrid sharding).

reference math:
    K = einsum('kd,hzd->khz', g, Wk); Q = einsum('qd,hzd->qhz', g, Wq)
    scores = beta * einsum('qhz,khz->hqk', Q, K)        # [H, N, N]
    A = logsumexp(scores, -1); out = (-1/beta) * A.sum()

Sharding (no collectives; final scalar reduction on host):
    core c owns head A = c (all 2048 q rows) and head B = 8 + c//2
    restricted to q rows [1024*(c%2), 1024*(c%2)+1024).  Every core runs an
    identical program; only input values differ (SPMD-safe).

Implementation notes:
  - inputs are cast to bf16 on the host (beta folded into Wq); matmuls are
    bf16 with fp32 PSUM accumulation
  - g -> gT rides the DMA crossbar transpose straight out of DRAM (the xbar
    is the serial startup resource, ~261 GB/s); W and gb transpose on the
    otherwise-idle PE via identity matmuls.  All DMA stays on the sync ring:
    concurrent xbar-transpose + regular DMA on different rings hangs TRN2.
  - scores use ONE psum pool of four [128,1024] half-tiles (8 banks) so the
    PE->max->exp chain pipelines 4 deep at half-unit granularity; per unit:
    2 half reduce_maxes -> combine -> 2 ACT exp(bias=-m) with fused row-sums
  - host finishes: A = m + log(l0 + l1), fp64 sum, scale by -1/beta
"""

import numpy as np
import ml_dtypes
from contextlib import ExitStack

import concourse.bass as bass
import concourse.mybir as mybir
import concourse.tile as tile
from concourse import bacc
from concourse.bass_utils import run_bass_kernel_spmd
from concourse.masks import make_identity

N, D, H, Y = 2048, 768, 12, 64
NCORES = 8
BETA = 1.0 / 8.0
DT = mybir.dt.float32
DTB = mybir.dt.bfloat16


def _units():
    # all A-units first: QT_B/KT_B operands land late (gb is DMA'd after the
    # g xbar transposes), and PE row-group packing is not the binding engine
    return [("A", j) for j in range(16)] + [("B", j) for j in range(8)]


def _build_kernel():
    nc = bacc.Bacc("TRN2", target_bir_lowering=False, debug=False, num_devices=1)
    g_ap = nc.dram_tensor("g", [N, D], DTB, kind="ExternalInput").ap()
    gb_ap = nc.dram_tensor("gb", [N // 2, D], DTB, kind="ExternalInput").ap()
    wq_ap = nc.dram_tensor("wq", [128, D], DTB, kind="ExternalInput").ap()
    wk_ap = nc.dram_tensor("wk", [128, D], DTB, kind="ExternalInput").ap()
    out_ap = nc.dram_tensor("stats", [128, 24 * 4], DT, kind="ExternalOutput").ap()

    AF = mybir.ActivationFunctionType
    AX = mybir.AxisListType
    OP = mybir.AluOpType

    with tile.TileContext(nc) as tc, ExitStack() as ctx:
        const_pool = ctx.enter_context(tc.tile_pool(name="const", bufs=1))
        ident = const_pool.tile([128, 128], DTB)
        make_identity(nc, ident[:])

        w_pool = ctx.enter_context(tc.tile_pool(name="w", bufs=1))
        wq_sb = w_pool.tile([128, D], DTB)
        nc.sync.dma_start(wq_sb[:], wq_ap[:])
        wk_sb = w_pool.tile([128, D], DTB)
        nc.sync.dma_start(wk_sb[:], wk_ap[:])
        # wt blocks 0..5 = WqT d-tiles ([128 d, 64 zA | 64 zB]), 6..11 = WkT
        wt_sb = w_pool.tile([128, 12 * 128], DTB)

        proj_pool = ctx.enter_context(tc.tile_pool(name="proj", bufs=1))
        kt_sb = proj_pool.tile([128, N], DTB)       # rows 0:64 KT_A, 64:128 KT_B
        qta_sb = proj_pool.tile([64, N], DTB)       # QT of head A, all q
        qtb_sb = proj_pool.tile([128, N // 2], DTB)  # rows 64:128 = QT of head B
        stat_pool = ctx.enter_context(tc.tile_pool(name="stat", bufs=8))

        # scores pool: three [128,1024] fp32 slots (6 banks); aux pool for
        # transposes/projections: two [128,512] slots (2 banks)
        pp = ctx.enter_context(tc.tile_pool(name="pp", bufs=3, space="PSUM"))
        aux = ctx.enter_context(tc.tile_pool(name="aux", bufs=2, space="PSUM"))

        gsrc_pool = ctx.enter_context(tc.tile_pool(name="gsrc", bufs=1))
        gt_pool = ctx.enter_context(tc.tile_pool(name="gt", bufs=1))

        # ---- g -> gT via xbar transpose straight from DRAM (sync ring):
        # gt[c][p, t, i] = g[512c + i, 128t + p]
        gt = []
        for c in range(4):
            gtc = gt_pool.tile([128, 6, 512], DTB, name=f"gt{c}")
            nc.sync.dma_start_transpose(gtc[:], g_ap[512 * c : 512 * (c + 1), :])
            gt.append(gtc)

        # gb regular DMA, after the transposes on the same ring
        gb_sb = gsrc_pool.tile([128, 8, D], DTB)
        gb_r = gb_ap.rearrange("(i p) d -> p i d", p=128)
        for c in range(2):
            nc.sync.dma_start(
                gb_sb[:, 4 * c : 4 * (c + 1), :], gb_r[:, 4 * c : 4 * (c + 1), :]
            )

        gtb_sb = gt_pool.tile([128, 6, N // 2], DTB)

        # ---- W transposes on PE: 12 [128,128] blocks, 4 per aux slot
        for grp in range(3):
            ps = aux.tile([128, 512], DTB, tag="x", name="ps_w")
            for j in range(4):
                blk = grp * 4 + j
                src = wq_sb if blk < 6 else wk_sb
                t = blk % 6
                nc.tensor.transpose(
                    ps[:, 128 * j : 128 * (j + 1)],
                    src[:, 128 * t : 128 * (t + 1)],
                    ident[:],
                )
            nc.vector.tensor_copy(wt_sb[:, 512 * grp : 512 * (grp + 1)], ps[:])

        # ---- projection helpers (aux pool, one 512-chunk per slot)
        def emit_kt(c):
            ps = aux.tile([128, 512], DT, tag="x", name="ps_kt")
            for t in range(6):
                nc.tensor.matmul(
                    ps[:],
                    lhsT=wt_sb[:, 128 * (6 + t) : 128 * (7 + t)],
                    rhs=gt[c][:, t, :],
                    start=(t == 0),
                    stop=(t == 5),
                )
            nc.scalar.copy(kt_sb[:, 512 * c : 512 * (c + 1)], ps[:])

        def emit_qta(c):
            ps = aux.tile([128, 512], DT, tag="x", name="ps_qta")[0:64, :]
            for t in range(6):
                nc.tensor.matmul(
                    ps[:],
                    lhsT=wt_sb[:, 128 * t : 128 * t + 64],
                    rhs=gt[c][:, t, :],
                    start=(t == 0),
                    stop=(t == 5),
                )
            nc.scalar.copy(qta_sb[:, 512 * c : 512 * (c + 1)], ps[:])

        # ---- score half-jobs: each [128,1024] half has its OWN neg-max and
        # exp row-sum; the host merges the two half-LSEs of a unit exactly.
        # stats layout: out[:, 4u + 2h] = neg_m, out[:, 4u + 2h + 1] = l
        def emit_half(u, kind, j, h):
            st = stat_pool.tile([128, 2], DT, tag="st", name="st")
            ps = pp.tile([128, 1024], DT, tag="h", name="ps_s")
            for sub in range(2):
                c = 2 * h + sub
                if kind == "A":
                    lhsT = qta_sb[:, 128 * j : 128 * (j + 1)]
                    rhs = kt_sb[0:64, 512 * c : 512 * (c + 1)]
                else:
                    lhsT = qtb_sb[64:128, 128 * j : 128 * (j + 1)]
                    rhs = kt_sb[64:128, 512 * c : 512 * (c + 1)]
                nc.tensor.matmul(
                    ps[:, 512 * sub : 512 * (sub + 1)],
                    lhsT=lhsT,
                    rhs=rhs,
                    start=True,
                    stop=True,
                )
            nc.vector.tensor_reduce(
                st[:, 0:1], ps[:], axis=AX.X, op=OP.max, negate=True
            )
            nc.scalar.activation(
                ps[:], ps[:], AF.Exp, bias=st[:, 0:1], scale=1.0,
                accum_out=st[:, 1:2],
            )
            nc.sync.dma_start(out_ap[:, 4 * u + 2 * h : 4 * u + 2 * h + 2], st[:])

        # ---- emission order = PE program order.  h0 halves need kt chunks
        # 0,1 only; h1 need chunks 2,3 (ready after the last xbar transpose).
        emit_kt(0)
        emit_kt(1)
        emit_qta(0)
        for j in range(4):
            emit_half(j, "A", j, 0)
        emit_qta(1)
        emit_kt(2)
        emit_kt(3)
        for j in range(4, 8):
            emit_half(j, "A", j, 0)
        emit_qta(2)
        emit_qta(3)
        for j in range(8, 16):
            emit_half(j, "A", j, 0)
        for j in range(16):
            emit_half(j, "A", j, 1)

        # ---- gb transposes on PE: 4 blocks per aux slot (gb has long arrived)
        for t in range(6):
            for c in range(2):
                ps = aux.tile([128, 512], DTB, tag="x", name="ps_gb")
                for jj in range(4):
                    i = 4 * c + jj
                    nc.tensor.transpose(
                        ps[:, 128 * jj : 128 * (jj + 1)],
                        gb_sb[:, i, 128 * t : 128 * (t + 1)],
                        ident[:],
                    )
                nc.vector.tensor_copy(gtb_sb[:, t, 512 * c : 512 * (c + 1)], ps[:])

        # QT of head B at partitions 0:64, then DMA-shift to 64:128
        qtb_lo = proj_pool.tile([64, N // 2], DTB)
        for c in range(2):
            ps = aux.tile([128, 512], DT, tag="x", name="ps_qtb")[0:64, :]
            for t in range(6):
                nc.tensor.matmul(
                    ps[:],
                    lhsT=wt_sb[:, 128 * t + 64 : 128 * (t + 1)],
                    rhs=gtb_sb[:, t, 512 * c : 512 * (c + 1)],
                    start=(t == 0),
                    stop=(t == 5),
                )
            nc.scalar.copy(qtb_lo[:, 512 * c : 512 * (c + 1)], ps[:])
        nc.sync.dma_start(qtb_sb[64:128, :], qtb_lo[:])

        for j in range(8):
            emit_half(16 + j, "B", j, 0)
        for j in range(8):
            emit_half(16 + j, "B", j, 1)

    nc.compile()
    return nc


_NC_CACHE = {}


def _get_nc():
    if "nc" not in _NC_CACHE:
        _NC_CACHE["nc"] = _build_kernel()
    return _NC_CACHE["nc"]


def _make_in_maps(np_inputs):
    bf16 = ml_dtypes.bfloat16
    g = np.ascontiguousarray(np.asarray(np_inputs["g"], dtype=np.float32).astype(bf16))
    Wq = np.asarray(np_inputs["Wq"], dtype=np.float32) * np.float32(BETA)
    Wk = np.asarray(np_inputs["Wk"], dtype=np.float32)
    in_maps = []
    for c in range(NCORES):
        hb = 8 + c // 2
        qlo = (N // 2) * (c % 2)
        in_maps.append(
            {
                "g": g,
                "gb": np.ascontiguousarray(g[qlo : qlo + N // 2]),
                "wq": np.ascontiguousarray(
                    np.concatenate([Wq[c], Wq[hb]], axis=0).astype(bf16)
                ),
                "wk": np.ascontiguousarray(
                    np.concatenate([Wk[c], Wk[hb]], axis=0).astype(bf16)
                ),
            }
        )
    return in_maps


def kernel(g, Wq, Wk):
    in_maps = _make_in_maps({"g": g, "Wq": Wq, "Wk": Wk})
    nc = _get_nc()
    res = run_bass_kernel_spmd(nc, in_maps, core_ids=list(range(NCORES)))

    total = 0.0
    for c in range(NCORES):
        stats = res.results[c]["stats"].astype(np.float64)  # [128, 96]
        m0 = -stats[:, 0::4]
        l0 = stats[:, 1::4]
        m1 = -stats[:, 2::4]
        l1 = stats[:, 3::4]
        m = np.maximum(m0, m1)
        l = l0 * np.exp(m0 - m) + l1 * np.exp(m1 - m)
        total += (m + np.log(l)).sum()
    return np.float32(-(1.0 / BETA) * total)


# revision 20
# speedup vs baseline: 1.0367x; 1.0367x over previous
"""EnergyAttention Trainium2 kernel (8-core SPMD, head/q hybrid sharding).

reference math:
    K = einsum('kd,hzd->khz', g, Wk); Q = einsum('qd,hzd->qhz', g, Wq)
    scores = beta * einsum('qhz,khz->hqk', Q, K)        # [H, N, N]
    A = logsumexp(scores, -1); out = (-1/beta) * A.sum()

Sharding (no collectives; final scalar reduction on host):
    core c owns head A = c (all 2048 q rows) and head B = 8 + c//2
    restricted to q rows [1024*(c%2), 1024*(c%2)+1024).  Every core runs an
    identical program; only input values differ (SPMD-safe).

Implementation notes:
  - inputs are cast to bf16 on the host (beta folded into Wq); matmuls are
    bf16 with fp32 PSUM accumulation
  - g -> gT rides the DMA crossbar transpose straight out of DRAM (the xbar
    is the serial startup resource, ~261 GB/s); W and gb transpose on the
    otherwise-idle PE via identity matmuls.  All DMA stays on the sync ring:
    concurrent xbar-transpose + regular DMA on different rings hangs TRN2.
  - scores use ONE psum pool of four [128,1024] half-tiles (8 banks) so the
    PE->max->exp chain pipelines 4 deep at half-unit granularity; per unit:
    2 half reduce_maxes -> combine -> 2 ACT exp(bias=-m) with fused row-sums
  - host finishes: A = m + log(l0 + l1), fp64 sum, scale by -1/beta
"""

import numpy as np
import ml_dtypes
from contextlib import ExitStack

import concourse.bass as bass
import concourse.mybir as mybir
import concourse.tile as tile
from concourse import bacc
from concourse.bass_utils import run_bass_kernel_spmd
from concourse.masks import make_identity

N, D, H, Y = 2048, 768, 12, 64
NCORES = 8
BETA = 1.0 / 8.0
DT = mybir.dt.float32
DTB = mybir.dt.bfloat16


def _units():
    # all A-units first: QT_B/KT_B operands land late (gb is DMA'd after the
    # g xbar transposes), and PE row-group packing is not the binding engine
    return [("A", j) for j in range(16)] + [("B", j) for j in range(8)]


def _build_kernel():
    nc = bacc.Bacc("TRN2", target_bir_lowering=False, debug=False, num_devices=1)
    g_ap = nc.dram_tensor("g", [N, D], DTB, kind="ExternalInput").ap()
    gb_ap = nc.dram_tensor("gb", [N // 2, D], DTB, kind="ExternalInput").ap()
    wq_ap = nc.dram_tensor("wq", [128, D], DTB, kind="ExternalInput").ap()
    wk_ap = nc.dram_tensor("wk", [128, D], DTB, kind="ExternalInput").ap()
    out_ap = nc.dram_tensor("stats", [128, 24 * 4], DT, kind="ExternalOutput").ap()

    AF = mybir.ActivationFunctionType
    AX = mybir.AxisListType
    OP = mybir.AluOpType

    with tile.TileContext(nc) as tc, ExitStack() as ctx:
        const_pool = ctx.enter_context(tc.tile_pool(name="const", bufs=1))
        ident = const_pool.tile([128, 128], DTB)
        make_identity(nc, ident[:])

        w_pool = ctx.enter_context(tc.tile_pool(name="w", bufs=1))
        wq_sb = w_pool.tile([128, D], DTB)
        nc.sync.dma_start(wq_sb[:], wq_ap[:])
        wk_sb = w_pool.tile([128, D], DTB)
        nc.sync.dma_start(wk_sb[:], wk_ap[:])
        # wt blocks 0..5 = WqT d-tiles ([128 d, 64 zA | 64 zB]), 6..11 = WkT
        wt_sb = w_pool.tile([128, 12 * 128], DTB)

        proj_pool = ctx.enter_context(tc.tile_pool(name="proj", bufs=1))
        kt_sb = proj_pool.tile([128, N], DTB)       # rows 0:64 KT_A, 64:128 KT_B
        qta_sb = proj_pool.tile([64, N], DTB)       # QT of head A, all q
        qtb_sb = proj_pool.tile([128, N // 2], DTB)  # rows 64:128 = QT of head B
        stat_pool = ctx.enter_context(tc.tile_pool(name="stat", bufs=8))

        # scores pool: three [128,1024] fp32 slots (6 banks); aux pool for
        # transposes/projections: two [128,512] slots (2 banks)
        pp = ctx.enter_context(tc.tile_pool(name="pp", bufs=3, space="PSUM"))
        aux = ctx.enter_context(tc.tile_pool(name="aux", bufs=2, space="PSUM"))

        gsrc_pool = ctx.enter_context(tc.tile_pool(name="gsrc", bufs=1))
        gt_pool = ctx.enter_context(tc.tile_pool(name="gt", bufs=1))

        # ---- g -> gT via xbar transpose straight from DRAM (sync ring):
        # gt[c][p, t, i] = g[512c + i, 128t + p]
        gt = []
        for c in range(4):
            gtc = gt_pool.tile([128, 6, 512], DTB, name=f"gt{c}")
            nc.sync.dma_start_transpose(gtc[:], g_ap[512 * c : 512 * (c + 1), :])
            gt.append(gtc)

        # gb regular DMA, after the transposes on the same ring
        gb_sb = gsrc_pool.tile([128, 8, D], DTB)
        gb_r = gb_ap.rearrange("(i p) d -> p i d", p=128)
        for c in range(2):
            nc.sync.dma_start(
                gb_sb[:, 4 * c : 4 * (c + 1), :], gb_r[:, 4 * c : 4 * (c + 1), :]
            )

        gtb_sb = gt_pool.tile([128, 6, N // 2], DTB)

        # ---- W transposes on PE: 12 [128,128] blocks, 4 per aux slot
        for grp in range(3):
            ps = aux.tile([128, 512], DTB, tag="x", name="ps_w")
            for j in range(4):
                blk = grp * 4 + j
                src = wq_sb if blk < 6 else wk_sb
                t = blk % 6
                nc.tensor.transpose(
                    ps[:, 128 * j : 128 * (j + 1)],
                    src[:, 128 * t : 128 * (t + 1)],
                    ident[:],
                )
            nc.vector.tensor_copy(wt_sb[:, 512 * grp : 512 * (grp + 1)], ps[:])

        # ---- projection helpers (aux pool, one 512-chunk per slot)
        def emit_kt(c):
            ps = aux.tile([128, 512], DT, tag="x", name="ps_kt")
            for t in range(6):
                nc.tensor.matmul(
                    ps[:],
                    lhsT=wt_sb[:, 128 * (6 + t) : 128 * (7 + t)],
                    rhs=gt[c][:, t, :],
                    start=(t == 0),
                    stop=(t == 5),
                )
            nc.scalar.copy(kt_sb[:, 512 * c : 512 * (c + 1)], ps[:])

        def emit_qta(c):
            ps = aux.tile([128, 512], DT, tag="x", name="ps_qta")[0:64, :]
            for t in range(6):
                nc.tensor.matmul(
                    ps[:],
                    lhsT=wt_sb[:, 128 * t : 128 * t + 64],
                    rhs=gt[c][:, t, :],
                    start=(t == 0),
                    stop=(t == 5),
                )
            nc.scalar.copy(qta_sb[:, 512 * c : 512 * (c + 1)], ps[:])

        # ---- score half-jobs: each [128,1024] half has its OWN neg-max and
        # exp row-sum; the host merges the two half-LSEs of a unit exactly.
        # stats layout: out[:, 4u + 2h] = neg_m, out[:, 4u + 2h + 1] = l
        def emit_half(u, kind, j, h):
            st = stat_pool.tile([128, 2], DT, tag="st", name="st")
            ps = pp.tile([128, 1024], DT, tag="h", name="ps_s")
            for sub in range(2):
                c = 2 * h + sub
                if kind == "A":
                    lhsT = qta_sb[:, 128 * j : 128 * (j + 1)]
                    rhs = kt_sb[0:64, 512 * c : 512 * (c + 1)]
                else:
                    lhsT = qtb_sb[64:128, 128 * j : 128 * (j + 1)]
                    rhs = kt_sb[64:128, 512 * c : 512 * (c + 1)]
                nc.tensor.matmul(
                    ps[:, 512 * sub : 512 * (sub + 1)],
                    lhsT=lhsT,
                    rhs=rhs,
                    start=True,
                    stop=True,
                )
            nc.vector.tensor_reduce(
                st[:, 0:1], ps[:], axis=AX.X, op=OP.max, negate=True
            )
            nc.scalar.activation(
                ps[:], ps[:], AF.Exp, bias=st[:, 0:1], scale=1.0,
                accum_out=st[:, 1:2],
            )
            nc.sync.dma_start(out_ap[:, 4 * u + 2 * h : 4 * u + 2 * h + 2], st[:])

        # gb-transpose / QTB emission pieces, interleaved into the A-half
        # stream so the B operands are ready (and their ACT copies queued)
        # well before the B halves, without a PE bubble.
        def emit_gb_piece(k):
            t, c = divmod(k, 2)
            ps = aux.tile([128, 512], DTB, tag="x", name="ps_gb")
            for jj in range(4):
                i = 4 * c + jj
                nc.tensor.transpose(
                    ps[:, 128 * jj : 128 * (jj + 1)],
                    gb_sb[:, i, 128 * t : 128 * (t + 1)],
                    ident[:],
                )
            nc.vector.tensor_copy(gtb_sb[:, t, 512 * c : 512 * (c + 1)], ps[:])

        qtb_lo = proj_pool.tile([64, N // 2], DTB)

        def emit_qtb_piece(c):
            ps = aux.tile([128, 512], DT, tag="x", name="ps_qtb")[0:64, :]
            for t in range(6):
                nc.tensor.matmul(
                    ps[:],
                    lhsT=wt_sb[:, 128 * t + 64 : 128 * (t + 1)],
                    rhs=gtb_sb[:, t, 512 * c : 512 * (c + 1)],
                    start=(t == 0),
                    stop=(t == 5),
                )
            nc.scalar.copy(qtb_lo[:, 512 * c : 512 * (c + 1)], ps[:])

        # ---- emission order = PE program order.  h0 halves need kt chunks
        # 0,1 only; h1 need chunks 2,3 (ready after the last xbar transpose).
        emit_kt(0)
        emit_kt(1)
        emit_qta(0)
        for j in range(4):
            emit_half(j, "A", j, 0)
        emit_qta(1)
        emit_kt(2)
        emit_kt(3)
        for j in range(4, 8):
            emit_half(j, "A", j, 0)
            emit_gb_piece(j - 4)
        emit_qta(2)
        emit_qta(3)
        for j in range(8, 16):
            emit_half(j, "A", j, 0)
            emit_gb_piece(j - 4)
        for j in range(16):
            emit_half(j, "A", j, 1)
            if j == 0:
                emit_qtb_piece(0)
            elif j == 1:
                emit_qtb_piece(1)
            elif j == 2:
                nc.sync.dma_start(qtb_sb[64:128, :], qtb_lo[:])

        for j in range(8):
            emit_half(16 + j, "B", j, 0)
        for j in range(8):
            emit_half(16 + j, "B", j, 1)

    nc.compile()
    return nc


_NC_CACHE = {}


def _get_nc():
    if "nc" not in _NC_CACHE:
        _NC_CACHE["nc"] = _build_kernel()
    return _NC_CACHE["nc"]


def _make_in_maps(np_inputs):
    bf16 = ml_dtypes.bfloat16
    g = np.ascontiguousarray(np.asarray(np_inputs["g"], dtype=np.float32).astype(bf16))
    Wq = np.asarray(np_inputs["Wq"], dtype=np.float32) * np.float32(BETA)
    Wk = np.asarray(np_inputs["Wk"], dtype=np.float32)
    in_maps = []
    for c in range(NCORES):
        hb = 8 + c // 2
        qlo = (N // 2) * (c % 2)
        in_maps.append(
            {
                "g": g,
                "gb": np.ascontiguousarray(g[qlo : qlo + N // 2]),
                "wq": np.ascontiguousarray(
                    np.concatenate([Wq[c], Wq[hb]], axis=0).astype(bf16)
                ),
                "wk": np.ascontiguousarray(
                    np.concatenate([Wk[c], Wk[hb]], axis=0).astype(bf16)
                ),
            }
        )
    return in_maps


def kernel(g, Wq, Wk):
    in_maps = _make_in_maps({"g": g, "Wq": Wq, "Wk": Wk})
    nc = _get_nc()
    res = run_bass_kernel_spmd(nc, in_maps, core_ids=list(range(NCORES)))

    total = 0.0
    for c in range(NCORES):
        stats = res.results[c]["stats"].astype(np.float64)  # [128, 96]
        m0 = -stats[:, 0::4]
        l0 = stats[:, 1::4]
        m1 = -stats[:, 2::4]
        l1 = stats[:, 3::4]
        m = np.maximum(m0, m1)
        l = l0 * np.exp(m0 - m) + l1 * np.exp(m1 - m)
        total += (m + np.log(l)).sum()
    return np.float32(-(1.0 / BETA) * total)


# revision 21
# speedup vs baseline: 1.0810x; 1.0427x over previous
"""EnergyAttention Trainium2 kernel (8-core SPMD, head/q hybrid sharding).

reference math:
    K = einsum('kd,hzd->khz', g, Wk); Q = einsum('qd,hzd->qhz', g, Wq)
    scores = beta * einsum('qhz,khz->hqk', Q, K)        # [H, N, N]
    A = logsumexp(scores, -1); out = (-1/beta) * A.sum()

Sharding (no collectives; final scalar reduction on host):
    core c owns head A = c (all 2048 q rows) and head B = 8 + c//2
    restricted to q rows [1024*(c%2), 1024*(c%2)+1024).  Every core runs an
    identical program; only input values differ (SPMD-safe).

Implementation notes:
  - inputs are cast to bf16 on the host (beta folded into Wq); matmuls are
    bf16 with fp32 PSUM accumulation
  - g -> gT rides the DMA crossbar transpose straight out of DRAM (the xbar
    is the serial startup resource, ~261 GB/s); W and gb transpose on the
    otherwise-idle PE via identity matmuls.  All DMA stays on the sync ring:
    concurrent xbar-transpose + regular DMA on different rings hangs TRN2.
  - scores: three [128,1024] PSUM slots (6 banks) + a 2-bank aux pool; each
    1024-wide score half is an independent job (2 matmuls -> DVE
    reduce_max(negate) -> ACT exp(bias=-m_half) with fused accum row-sum), so
    the PE->max->exp chain pipelines 3 deep with no cross-half combine
  - host merges the half-LSEs exactly (l0*e^(m0-m) + l1*e^(m1-m)), sums in
    fp64 and scales by -1/beta
"""

import numpy as np
import ml_dtypes
from contextlib import ExitStack

import concourse.bass as bass
import concourse.mybir as mybir
import concourse.tile as tile
from concourse import bacc
from concourse.bass_utils import run_bass_kernel_spmd
from concourse.masks import make_identity

N, D, H, Y = 2048, 768, 12, 64
NCORES = 8
BETA = 1.0 / 8.0
DT = mybir.dt.float32
DTB = mybir.dt.bfloat16


def _units():
    # all A-units first: QT_B/KT_B operands land late (gb is DMA'd after the
    # g xbar transposes), and PE row-group packing is not the binding engine
    return [("A", j) for j in range(16)] + [("B", j) for j in range(8)]


def _build_kernel():
    nc = bacc.Bacc("TRN2", target_bir_lowering=False, debug=False, num_devices=1)
    g_ap = nc.dram_tensor("g", [N, D], DTB, kind="ExternalInput").ap()
    gb_ap = nc.dram_tensor("gb", [N // 2, D], DTB, kind="ExternalInput").ap()
    wq_ap = nc.dram_tensor("wq", [128, D], DTB, kind="ExternalInput").ap()
    wk_ap = nc.dram_tensor("wk", [128, D], DTB, kind="ExternalInput").ap()
    out_ap = nc.dram_tensor("stats", [128, 24 * 4], DT, kind="ExternalOutput").ap()

    AF = mybir.ActivationFunctionType
    AX = mybir.AxisListType
    OP = mybir.AluOpType

    with tile.TileContext(nc) as tc, ExitStack() as ctx:
        const_pool = ctx.enter_context(tc.tile_pool(name="const", bufs=1))
        ident = const_pool.tile([128, 128], DTB)
        make_identity(nc, ident[:])

        w_pool = ctx.enter_context(tc.tile_pool(name="w", bufs=1))
        wq_sb = w_pool.tile([128, D], DTB)
        nc.sync.dma_start(wq_sb[:], wq_ap[:])
        wk_sb = w_pool.tile([128, D], DTB)
        nc.sync.dma_start(wk_sb[:], wk_ap[:])
        # wt blocks 0..5 = WqT d-tiles ([128 d, 64 zA | 64 zB]), 6..11 = WkT
        wt_sb = w_pool.tile([128, 12 * 128], DTB)

        proj_pool = ctx.enter_context(tc.tile_pool(name="proj", bufs=1))
        kt_sb = proj_pool.tile([128, N], DTB)       # rows 0:64 KT_A, 64:128 KT_B
        qta_sb = proj_pool.tile([64, N], DTB)       # QT of head A, all q
        qtb_sb = proj_pool.tile([128, N // 2], DTB)  # rows 64:128 = QT of head B
        stat_pool = ctx.enter_context(tc.tile_pool(name="stat", bufs=8))

        # scores pool: three [128,1024] fp32 slots (6 banks); aux pool for
        # transposes/projections: two [128,512] slots (2 banks)
        pp = ctx.enter_context(tc.tile_pool(name="pp", bufs=3, space="PSUM"))
        aux = ctx.enter_context(tc.tile_pool(name="aux", bufs=2, space="PSUM"))

        gsrc_pool = ctx.enter_context(tc.tile_pool(name="gsrc", bufs=1))
        gt_pool = ctx.enter_context(tc.tile_pool(name="gt", bufs=1))

        # ---- g -> gT via xbar transpose straight from DRAM (sync ring):
        # gt[c][p, t, i] = g[512c + i, 128t + p]
        gt = []
        for c in range(4):
            gtc = gt_pool.tile([128, 6, 512], DTB, name=f"gt{c}")
            nc.sync.dma_start_transpose(gtc[:], g_ap[512 * c : 512 * (c + 1), :])
            gt.append(gtc)

        # gb regular DMA, after the transposes on the same ring
        gb_sb = gsrc_pool.tile([128, 8, D], DTB)
        gb_r = gb_ap.rearrange("(i p) d -> p i d", p=128)
        for c in range(2):
            nc.sync.dma_start(
                gb_sb[:, 4 * c : 4 * (c + 1), :], gb_r[:, 4 * c : 4 * (c + 1), :]
            )

        gtb_sb = gt_pool.tile([128, 6, N // 2], DTB)

        # ---- W transposes on PE: 12 [128,128] blocks, 4 per aux slot
        for grp in range(3):
            ps = aux.tile([128, 512], DTB, tag="x", name="ps_w")
            for j in range(4):
                blk = grp * 4 + j
                src = wq_sb if blk < 6 else wk_sb
                t = blk % 6
                nc.tensor.transpose(
                    ps[:, 128 * j : 128 * (j + 1)],
                    src[:, 128 * t : 128 * (t + 1)],
                    ident[:],
                )
            nc.vector.tensor_copy(wt_sb[:, 512 * grp : 512 * (grp + 1)], ps[:])

        # ---- projection helpers (aux pool, one 512-chunk per slot)
        def emit_kt(c):
            ps = aux.tile([128, 512], DT, tag="x", name="ps_kt")
            for t in range(6):
                nc.tensor.matmul(
                    ps[:],
                    lhsT=wt_sb[:, 128 * (6 + t) : 128 * (7 + t)],
                    rhs=gt[c][:, t, :],
                    start=(t == 0),
                    stop=(t == 5),
                )
            nc.scalar.copy(kt_sb[:, 512 * c : 512 * (c + 1)], ps[:])

        def emit_qta(c):
            ps = aux.tile([128, 512], DT, tag="x", name="ps_qta")[0:64, :]
            for t in range(6):
                nc.tensor.matmul(
                    ps[:],
                    lhsT=wt_sb[:, 128 * t : 128 * t + 64],
                    rhs=gt[c][:, t, :],
                    start=(t == 0),
                    stop=(t == 5),
                )
            nc.scalar.copy(qta_sb[:, 512 * c : 512 * (c + 1)], ps[:])

        # ---- score half-jobs: each [128,1024] half has its OWN neg-max and
        # exp row-sum; the host merges the two half-LSEs of a unit exactly.
        # stats layout: out[:, 4u + 2h] = neg_m, out[:, 4u + 2h + 1] = l
        def emit_half(u, kind, j, h):
            st = stat_pool.tile([128, 2], DT, tag="st", name="st")
            ps = pp.tile([128, 1024], DT, tag="h", name="ps_s")
            for sub in range(2):
                c = 2 * h + sub
                if kind == "A":
                    lhsT = qta_sb[:, 128 * j : 128 * (j + 1)]
                    rhs = kt_sb[0:64, 512 * c : 512 * (c + 1)]
                else:
                    lhsT = qtb_sb[64:128, 128 * j : 128 * (j + 1)]
                    rhs = kt_sb[64:128, 512 * c : 512 * (c + 1)]
                nc.tensor.matmul(
                    ps[:, 512 * sub : 512 * (sub + 1)],
                    lhsT=lhsT,
                    rhs=rhs,
                    start=True,
                    stop=True,
                )
            nc.vector.tensor_reduce(
                st[:, 0:1], ps[:], axis=AX.X, op=OP.max, negate=True
            )
            nc.scalar.activation(
                ps[:], ps[:], AF.Exp, bias=st[:, 0:1], scale=1.0,
                accum_out=st[:, 1:2],
            )
            nc.sync.dma_start(out_ap[:, 4 * u + 2 * h : 4 * u + 2 * h + 2], st[:])

        # gb-transpose / QTB emission pieces, interleaved into the A-half
        # stream so the B operands are ready (and their ACT copies queued)
        # well before the B halves, without a PE bubble.
        def emit_gb_piece(k):
            t, c = divmod(k, 2)
            ps = aux.tile([128, 512], DTB, tag="x", name="ps_gb")
            for jj in range(4):
                i = 4 * c + jj
                nc.tensor.transpose(
                    ps[:, 128 * jj : 128 * (jj + 1)],
                    gb_sb[:, i, 128 * t : 128 * (t + 1)],
                    ident[:],
                )
            nc.vector.tensor_copy(gtb_sb[:, t, 512 * c : 512 * (c + 1)], ps[:])

        qtb_lo = proj_pool.tile([64, N // 2], DTB)

        def emit_qtb_piece(c):
            ps = aux.tile([128, 512], DT, tag="x", name="ps_qtb")[0:64, :]
            for t in range(6):
                nc.tensor.matmul(
                    ps[:],
                    lhsT=wt_sb[:, 128 * t + 64 : 128 * (t + 1)],
                    rhs=gtb_sb[:, t, 512 * c : 512 * (c + 1)],
                    start=(t == 0),
                    stop=(t == 5),
                )
            nc.scalar.copy(qtb_lo[:, 512 * c : 512 * (c + 1)], ps[:])

        # ---- emission order = PE program order.  h0 halves need kt chunks
        # 0,1 only; h1 need chunks 2,3 (ready after the last xbar transpose).
        emit_kt(0)
        emit_kt(1)
        emit_qta(0)
        for j in range(4):
            emit_half(j, "A", j, 0)
        emit_qta(1)
        emit_kt(2)
        emit_kt(3)
        for j in range(4, 8):
            emit_half(j, "A", j, 0)
            emit_gb_piece(j - 4)
        emit_qta(2)
        emit_qta(3)
        for j in range(8, 16):
            emit_half(j, "A", j, 0)
            emit_gb_piece(j - 4)
        for j in range(16):
            emit_half(j, "A", j, 1)
            if j == 0:
                emit_qtb_piece(0)
            elif j == 1:
                emit_qtb_piece(1)
            elif j == 2:
                nc.sync.dma_start(qtb_sb[64:128, :], qtb_lo[:])

        for j in range(8):
            emit_half(16 + j, "B", j, 0)
        for j in range(8):
            emit_half(16 + j, "B", j, 1)

    nc.compile()
    return nc


_NC_CACHE = {}


def _get_nc():
    if "nc" not in _NC_CACHE:
        _NC_CACHE["nc"] = _build_kernel()
    return _NC_CACHE["nc"]


def _make_in_maps(np_inputs):
    bf16 = ml_dtypes.bfloat16
    g = np.ascontiguousarray(np.asarray(np_inputs["g"], dtype=np.float32).astype(bf16))
    Wq = np.asarray(np_inputs["Wq"], dtype=np.float32) * np.float32(BETA)
    Wk = np.asarray(np_inputs["Wk"], dtype=np.float32)
    in_maps = []
    for c in range(NCORES):
        hb = 8 + c // 2
        qlo = (N // 2) * (c % 2)
        in_maps.append(
            {
                "g": g,
                "gb": np.ascontiguousarray(g[qlo : qlo + N // 2]),
                "wq": np.ascontiguousarray(
                    np.concatenate([Wq[c], Wq[hb]], axis=0).astype(bf16)
                ),
                "wk": np.ascontiguousarray(
                    np.concatenate([Wk[c], Wk[hb]], axis=0).astype(bf16)
                ),
            }
        )
    return in_maps


def kernel(g, Wq, Wk):
    in_maps = _make_in_maps({"g": g, "Wq": Wq, "Wk": Wk})
    nc = _get_nc()
    res = run_bass_kernel_spmd(nc, in_maps, core_ids=list(range(NCORES)))

    total = 0.0
    for c in range(NCORES):
        stats = res.results[c]["stats"].astype(np.float64)  # [128, 96]
        m0 = -stats[:, 0::4]
        l0 = stats[:, 1::4]
        m1 = -stats[:, 2::4]
        l1 = stats[:, 3::4]
        m = np.maximum(m0, m1)
        l = l0 * np.exp(m0 - m) + l1 * np.exp(m1 - m)
        total += (m + np.log(l)).sum()
    return np.float32(-(1.0 / BETA) * total)
